# revision 11
# baseline (speedup 1.0000x reference)
"""DeepseekMoE block-quantized MoE kernel for 8 Trainium2 NeuronCores.

Strategy (expert-parallel with host-side dispatch):
  - The routing table (selected_experts) is known on the host before launch,
    so the all-to-all "dispatch" is done on the host: for each expert e we
    gather the unique tokens routed to it (dedup across the top-k slots),
    transpose to [H, n_e], and pad to a common capacity C.
  - Experts are sharded 2-per-core across the 8 cores.  Each core runs a
    dense 3-matmul MLP (gate/up -> silu*up -> down) for its 2 experts in
    x^T / act^T layout so no on-device transposes are needed.
  - Block-dequantization (w * repeat(s, 128)) is folded into the host-side
    weight preparation, which also emits slab-contiguous weight layouts so
    every weight DMA is a pure linear copy (4KB per partition line).
  - All matmul operands are bf16: same 1 col/cycle PE rate as fp32r, but
    half the HBM traffic (faster pipeline fill, no DMA-induced PE stalls)
    and FWL-eligible LDWEIGHTS (fp32 operands block fast weight load).
  - The host scatters the per-expert outputs back to [T, K, H].
"""

import math

import numpy as np

T = 4096
TOPK = 6
E = 16
H = 2048
I = 1408
BS = 128           # quant block size
HT = H // 128      # 16 h-tiles
IT = I // 128      # 11 i-tiles
NCORES = 8
# Single-pass width bound: at most 4 PSUM-bank chunks (the interleaved
# accumulation rings are 4 deep), and (HT + IT) * 2 * W bytes of x+act
# per partition plus ~40KB of staging must fit in ~208KB of SBUF.
MAX_W = 2040

_BUILT = {}
LAST_RESULTS = None  # stashed BassKernelResults for external harnesses


def _chunk_plan(width, small_first=False):
    """Split `width` columns into PSUM-bank-sized chunks (<=512), each >=256
    when width allows (small free dims pay LDWEIGHTS/dispatch overhead).
    With small_first, carve a 256-col leading chunk so the first matmul
    group's input DMA is small (faster pipeline fill at kernel start)."""
    if width <= 512:
        return [(0, width)]
    if small_first and width > 768:
        return [(0, 256)] + [(256 + o, w) for o, w in _chunk_plan(width - 256)]
    n = -(-width // 512)
    # 8-aligned chunk widths
    base = (width // n) // 8 * 8
    rem8 = (width - n * base) // 8
    out, off = [], 0
    for j in range(n):
        w = base + (8 if j < rem8 else 0)
        if j == n - 1:
            w = width - off
        out.append((off, w))
        off += w
    return out


def _build(jobs, CT):
    """Build the SPMD Bass program.  `jobs` is a tuple of
    (slot, col_offset, width): each job runs one expert slot's MLP over a
    window of `width` token columns; CT is the column capacity of xt/yt."""
    import concourse.bacc as bacc
    import concourse.mybir as mybir
    from concourse.bass import ts
    from concourse.tile import TileContext

    f32 = mybir.dt.float32
    bf16 = mybir.dt.bfloat16
    AF = mybir.ActivationFunctionType
    import os as _os

    act_fn = (
        AF.Sigmoid if _os.environ.get("KERNEL_SIM_SIGMOID") else AF.Silu
    )  # CoreSim lacks Silu; HW path always uses Silu

    nc = bacc.Bacc()
    xt = nc.declare_dram_parameter("xt", [2, HT, 128, CT], bf16, isOutput=False)
    # slab-contiguous weights: w0t/w1t slab i = [128, H]; w2t slab h = [128, I]
    w0t = nc.declare_dram_parameter("w0t", [2, IT, 128, H], bf16, isOutput=False)
    w1t = nc.declare_dram_parameter("w1t", [2, IT, 128, H], bf16, isOutput=False)
    w2t = nc.declare_dram_parameter("w2t", [2, HT, 128, I], bf16, isOutput=False)
    yt = nc.declare_dram_parameter("yt", [2, HT, 128, CT], bf16, isOutput=True)

    with TileContext(nc) as tc:
        with (
            tc.tile_pool(name="xp", bufs=1) as xp,
            tc.tile_pool(name="ap", bufs=1) as apool,
            tc.tile_pool(name="wp", bufs=2) as wp,
            tc.tile_pool(name="yp", bufs=4) as yp,
            tc.tile_pool(name="ps", bufs=3, space="PSUM") as ps,
        ):
            def load_w01_slab(which, src, s, i):
                slab = wp.tile([128, H], bf16, tag=which, name=None, bufs=3)
                nc.sync.dma_start(out=slab, in_=src[s, i])
                return slab

            for jn, (s, co, W) in enumerate(jobs):
                    chunks = _chunk_plan(W)
                    xs = [
                        xp.tile([128, W], bf16, tag=f"x{h}", name=f"x{h}_{jn}")
                        for h in range(HT)
                    ]
                    # Bandwidth-priority emission: weight slabs for i=0..2 on
                    # the sync queue, x h-tiles (full width, 2.8KB DMA lines)
                    # on the gpsimd queue so neither head-of-line blocks the
                    # other.  The PE starts once the i=0 gate slab and the
                    # first x h-tiles land.
                    slab_q = {0: [load_w01_slab("w0", w0t, s, 0)]}
                    for h in range(HT):
                        nc.gpsimd.dma_start(
                            out=xs[h], in_=xt[s, h, :, co : co + W]
                        )
                        if h == 0:
                            slab_q[0].append(load_w01_slab("w1", w1t, s, 0))
                        if h <= 1 and h + 1 < IT:
                            slab_q[h + 1] = [
                                load_w01_slab("w0", w0t, s, h + 1),
                                load_w01_slab("w1", w1t, s, h + 1),
                            ]
                    acts = [
                        apool.tile([128, W], bf16, tag=f"a{i}", name=f"a{i}_{jn}")
                        for i in range(IT)
                    ]

                    # Phase A: gate/up projections + silu*up, per i-tile.
                    # h-outer with chunk-interleaved PSUM accumulation: each
                    # xs[h] is consumed exactly once (streams at DMA delivery
                    # pace during the fill) and consecutive matmuls share the
                    # stationary weight tile.
                    n_pre = max(slab_q) + 1
                    for i in range(IT):
                        w0s, w1s = slab_q.pop(i)
                        i_next = i + n_pre
                        if i_next < IT:
                            slab_q[i_next] = [
                                load_w01_slab("w0", w0t, s, i_next),
                                load_w01_slab("w1", w1t, s, i_next),
                            ]
                        gb = [ps.tile([128, 512], f32, tag="g", bufs=4, name=f"g{jn}_{i}_{ci}") for ci in range(len(chunks))]
                        ub = [ps.tile([128, 512], f32, tag="u", bufs=4, name=f"u{jn}_{i}_{ci}") for ci in range(len(chunks))]
                        for h in range(HT):
                            for ci, (c0, cw) in enumerate(chunks):
                                nc.tensor.matmul(
                                    gb[ci][:, :cw],
                                    w0s[:, ts(h, 128)],
                                    xs[h][:, c0 : c0 + cw],
                                    start=(h == 0),
                                    stop=(h == HT - 1),
                                )
                            for ci, (c0, cw) in enumerate(chunks):
                                nc.tensor.matmul(
                                    ub[ci][:, :cw],
                                    w1s[:, ts(h, 128)],
                                    xs[h][:, c0 : c0 + cw],
                                    start=(h == 0),
                                    stop=(h == HT - 1),
                                )
                        for ci, (c0, cw) in enumerate(chunks):
                            a_sl = acts[i][:, c0 : c0 + cw]
                            nc.scalar.activation(a_sl, gb[ci][:, :cw], act_fn)
                            nc.vector.tensor_mul(a_sl, a_sl, ub[ci][:, :cw])

                    # Phase B: down projection, per h-tile, i-outer with the
                    # same chunk interleaving (PSUM banks shared with the "u"
                    # ring).  y is staged into a full-width SBUF row so the
                    # writeback is one DMA per h with 2.8KB partition lines.
                    for h in range(HT):
                        w2s = wp.tile([128, I], bf16, tag="w2", bufs=3)
                        nc.sync.dma_start(out=w2s, in_=w2t[s, h])
                        yc = yp.tile([128, W], bf16, tag="y", bufs=2)
                        ob = [ps.tile([128, 512], f32, tag="u", bufs=4, name=f"o{jn}_{h}_{ci}") for ci in range(len(chunks))]
                        for i in range(IT):
                            for ci, (c0, cw) in enumerate(chunks):
                                nc.tensor.matmul(
                                    ob[ci][:, :cw],
                                    w2s[:, ts(i, 128)],
                                    acts[i][:, c0 : c0 + cw],
                                    start=(i == 0),
                                    stop=(i == IT - 1),
                                )
                        for ci, (c0, cw) in enumerate(chunks):
                            nc.vector.tensor_copy(yc[:, c0 : c0 + cw], ob[ci][:, :cw])
                        nc.sync.dma_start(
                            out=yt[s, h, :, co : co + W], in_=yc
                        )
    nc.finalize()
    return nc


def _get_built(jobs, CT):
    key = (tuple(jobs), CT)
    if key not in _BUILT:
        _BUILT[key] = _build(tuple(jobs), CT)
    return _BUILT[key]


def _dequant(w, s):
    """w: [E, O, Iin], s: [E, O, Iin//128] -> dequantized [E, O, Iin]."""
    e, o, iin = w.shape
    nb = -(-iin // BS)
    if nb * BS != iin:
        s_full = np.repeat(s, BS, axis=-1)[..., :iin]
        return w * s_full
    return (w.reshape(e, o, nb, BS) * s[..., None]).reshape(e, o, iin)


def _slabify(wd, bf16):
    """wd: [E, O, C] dequantized weights -> [E, O//128, 128, C] bf16 where
    slab o = [128 c-sub partitions, O-tile columns grouped by c-tile]:
    out[e, o, p, ct*128+j] = wd[e, o*128+j, ct*128+p]."""
    e, o, c = wd.shape
    ot, ct = o // 128, c // 128
    v = wd.astype(bf16).reshape(e, ot, 128, ct, 128)
    return v.transpose(0, 1, 4, 3, 2).reshape(e, ot, 128, c)


def kernel(**inputs):
    global LAST_RESULTS
    import ml_dtypes

    bf16 = ml_dtypes.bfloat16

    x = np.ascontiguousarray(np.asarray(inputs["x"], dtype=np.float32))
    sel = np.asarray(inputs["selected_experts"])
    w0 = np.asarray(inputs["w0"], dtype=np.float32)
    s0 = np.asarray(inputs["s0"], dtype=np.float32)
    w1 = np.asarray(inputs["w1"], dtype=np.float32)
    s1 = np.asarray(inputs["s1"], dtype=np.float32)
    w2 = np.asarray(inputs["w2"], dtype=np.float32)
    s2 = np.asarray(inputs["s2"], dtype=np.float32)

    t, k = sel.shape
    assert (t, k) == (T, TOPK) and x.shape == (T, H)

    # ---- host-side dispatch: unique tokens per expert ----
    pos = np.full((E, T), -1, dtype=np.int32)
    cols = []
    for e in range(E):
        toks = np.nonzero((sel == e).any(axis=1))[0]
        cols.append(toks)
        pos[e, toks] = np.arange(len(toks), dtype=np.int32)
    counts = np.array([len(c) for c in cols])

    # Assign experts to (core, slot): slot 0 holds the 8 largest experts,
    # slot 1 the 8 smallest, so each slot's padded width is only the max of
    # its own rank group.  expert_of[s][c] = expert on core c, slot s.
    order = np.argsort(-counts, kind="stable")
    expert_of = [list(order[:NCORES]), list(order[NCORES:])]

    def align8(v):
        return max(256, -(-v // 4) * 4)

    slot_w = [align8(int(counts[expert_of[s]].max())) for s in range(2)]

    if max(slot_w) <= MAX_W:
        jobs = tuple((s, 0, slot_w[s]) for s in range(2))
        CT = max(slot_w)
    else:
        # fallback: uniform width, multiple column windows per slot
        cmax = int(counts.max())
        passes = max(1, math.ceil(cmax / MAX_W))
        W = align8(math.ceil(cmax / passes))
        CT = W * passes
        jobs = tuple((s, cp * W, W) for s in range(2) for cp in range(passes))

    # ---- dequantize + slabify weights (host) ----
    # w0/w1: [E, I, H] -> slabs [E, IT, 128, H]; w2: [E, H, I] -> [E, HT, 128, I]
    w0s_all = _slabify(_dequant(w0, s0), bf16)
    w1s_all = _slabify(_dequant(w1, s1), bf16)
    w2s_all = _slabify(_dequant(w2, s2), bf16)

    x_bf = x.astype(bf16)
    in_maps = []
    for c in range(NCORES):
        pair = [expert_of[0][c], expert_of[1][c]]
        xt_c = np.zeros((2, H, CT), dtype=bf16)
        for s, e in enumerate(pair):
            n = len(cols[e])
            if n:
                xt_c[s, :, :n] = x_bf[cols[e]].T
        in_maps.append(
            {
                "xt": xt_c.reshape(2, HT, 128, CT),
                "w0t": np.ascontiguousarray(w0s_all[pair]),
                "w1t": np.ascontiguousarray(w1s_all[pair]),
                "w2t": np.ascontiguousarray(w2s_all[pair]),
            }
        )

    nc = _get_built(jobs, CT)
    from concourse.bass_utils import run_bass_kernel_spmd

    res = run_bass_kernel_spmd(nc, in_maps, list(range(NCORES)))
    LAST_RESULTS = res

    # Y[e] = [H, CT] for expert e
    Y = np.empty((E, H, CT), dtype=np.float32)
    for c in range(NCORES):
        yt_c = np.asarray(res.results[c]["yt"]).astype(np.float32).reshape(2, H, CT)
        Y[expert_of[0][c]] = yt_c[0]
        Y[expert_of[1][c]] = yt_c[1]

    # ---- scatter back to [T, K, H] ----
    e_flat = sel.reshape(-1).astype(np.int64)
    t_flat = np.repeat(np.arange(T, dtype=np.int64), TOPK)
    p_flat = pos[e_flat, t_flat]
    out = Y[e_flat, :, p_flat]  # [T*K, H]
    return np.ascontiguousarray(out.reshape(T, TOPK, H), dtype=np.float32)


# revision 12
# speedup vs baseline: 1.0011x; 1.0011x over previous
"""DeepseekMoE block-quantized MoE kernel for 8 Trainium2 NeuronCores.

Strategy (expert-parallel with host-side dispatch):
  - The routing table (selected_experts) is known on the host before launch,
    so the all-to-all "dispatch" is done on the host: for each expert e we
    gather the unique tokens routed to it (dedup across the top-k slots),
    transpose to [H, n_e], and pad to a common capacity C.
  - Experts are sharded 2-per-core across the 8 cores.  Each core runs a
    dense 3-matmul MLP (gate/up -> silu*up -> down) for its 2 experts in
    x^T / act^T layout so no on-device transposes are needed.
  - Block-dequantization (w * repeat(s, 128)) is folded into the host-side
    weight preparation, which also emits slab-contiguous weight layouts so
    every weight DMA is a pure linear copy (4KB per partition line).
  - All matmul operands are bf16: same 1 col/cycle PE rate as fp32r, but
    half the HBM traffic (faster pipeline fill, no DMA-induced PE stalls)
    and FWL-eligible LDWEIGHTS (fp32 operands block fast weight load).
  - Loop order is h-outer with chunk-interleaved PSUM accumulation (4-deep
    g/u rings, 8 banks total), so each x h-tile is consumed exactly once at
    DMA delivery pace and consecutive matmuls share the stationary weights.
  - x loads ride the gpsimd queue, weight/output DMAs the sync queue, so
    a job's x prefetch never head-of-line blocks the running job's DMAs.
  - The host scatters the per-expert outputs back to [T, K, H].

  Measured on 8 trn2 cores: 631-633us vs a 594.9us compute-stream floor
  (2704 padded cols x 528 PE cycles/col @ 2.4 GHz); the gap is runtime
  preamble (~7us), bandwidth-structural pipeline fill, and drain tail.
  fp8 was evaluated and rejected: e4m3 activation quantization alone puts
  rel_l2 at ~4.6e-2, over the 2e-2 gate (bf16 lands at 4.7e-3).
"""

import math

import numpy as np

T = 4096
TOPK = 6
E = 16
H = 2048
I = 1408
BS = 128           # quant block size
HT = H // 128      # 16 h-tiles
IT = I // 128      # 11 i-tiles
NCORES = 8
# Single-pass width bound: at most 4 PSUM-bank chunks (the interleaved
# accumulation rings are 4 deep), and (HT + IT) * 2 * W bytes of x+act
# per partition plus ~40KB of staging must fit in ~208KB of SBUF.
MAX_W = 2040

_BUILT = {}
LAST_RESULTS = None  # stashed BassKernelResults for external harnesses


def _chunk_plan(width, small_first=False):
    """Split `width` columns into PSUM-bank-sized chunks (<=512), each >=256
    when width allows (small free dims pay LDWEIGHTS/dispatch overhead).
    With small_first, carve a 256-col leading chunk so the first matmul
    group's input DMA is small (faster pipeline fill at kernel start)."""
    if width <= 512:
        return [(0, width)]
    if small_first and width > 768:
        return [(0, 256)] + [(256 + o, w) for o, w in _chunk_plan(width - 256)]
    n = -(-width // 512)
    # 8-aligned chunk widths
    base = (width // n) // 8 * 8
    rem8 = (width - n * base) // 8
    out, off = [], 0
    for j in range(n):
        w = base + (8 if j < rem8 else 0)
        if j == n - 1:
            w = width - off
        out.append((off, w))
        off += w
    return out


def _build(jobs, CT):
    """Build the SPMD Bass program.  `jobs` is a tuple of
    (slot, col_offset, width): each job runs one expert slot's MLP over a
    window of `width` token columns; CT is the column capacity of xt/yt."""
    import concourse.bacc as bacc
    import concourse.mybir as mybir
    from concourse.bass import ts
    from concourse.tile import TileContext

    f32 = mybir.dt.float32
    bf16 = mybir.dt.bfloat16
    AF = mybir.ActivationFunctionType
    import os as _os

    act_fn = (
        AF.Sigmoid if _os.environ.get("KERNEL_SIM_SIGMOID") else AF.Silu
    )  # CoreSim lacks Silu; HW path always uses Silu

    nc = bacc.Bacc()
    xt = nc.declare_dram_parameter("xt", [2, HT, 128, CT], bf16, isOutput=False)
    # slab-contiguous weights: w0t/w1t slab i = [128, H]; w2t slab h = [128, I]
    w0t = nc.declare_dram_parameter("w0t", [2, IT, 128, H], bf16, isOutput=False)
    w1t = nc.declare_dram_parameter("w1t", [2, IT, 128, H], bf16, isOutput=False)
    w2t = nc.declare_dram_parameter("w2t", [2, HT, 128, I], bf16, isOutput=False)
    yt = nc.declare_dram_parameter("yt", [2, HT, 128, CT], bf16, isOutput=True)

    with TileContext(nc) as tc:
        with (
            tc.tile_pool(name="xp", bufs=1) as xp,
            tc.tile_pool(name="ap", bufs=1) as apool,
            tc.tile_pool(name="wp", bufs=2) as wp,
            tc.tile_pool(name="yp", bufs=4) as yp,
            tc.tile_pool(name="ps", bufs=3, space="PSUM") as ps,
        ):
            def load_w01_slab(which, src, s, i):
                slab = wp.tile([128, H], bf16, tag=which, name=None, bufs=3)
                nc.sync.dma_start(out=slab, in_=src[s, i])
                return slab

            for jn, (s, co, W) in enumerate(jobs):
                    chunks = _chunk_plan(W)
                    xs = [
                        xp.tile([128, W], bf16, tag=f"x{h}", name=f"x{h}_{jn}")
                        for h in range(HT)
                    ]
                    # Bandwidth-priority emission: weight slabs for i=0..2 on
                    # the sync queue, x h-tiles (full width, 2.8KB DMA lines)
                    # on the gpsimd queue so neither head-of-line blocks the
                    # other.  The PE starts once the i=0 gate slab and the
                    # first x h-tiles land.
                    slab_q = {0: [load_w01_slab("w0", w0t, s, 0)]}
                    for h in range(HT):
                        nc.gpsimd.dma_start(
                            out=xs[h], in_=xt[s, h, :, co : co + W]
                        )
                        if h == 0:
                            slab_q[0].append(load_w01_slab("w1", w1t, s, 0))
                        if h <= 1 and h + 1 < IT:
                            slab_q[h + 1] = [
                                load_w01_slab("w0", w0t, s, h + 1),
                                load_w01_slab("w1", w1t, s, h + 1),
                            ]
                    acts = [
                        apool.tile([128, W], bf16, tag=f"a{i}", name=f"a{i}_{jn}")
                        for i in range(IT)
                    ]

                    # Phase A: gate/up projections + silu*up, per i-tile.
                    # h-outer with chunk-interleaved PSUM accumulation: each
                    # xs[h] is consumed exactly once (streams at DMA delivery
                    # pace during the fill) and consecutive matmuls share the
                    # stationary weight tile.
                    n_pre = max(slab_q) + 1
                    for i in range(IT):
                        w0s, w1s = slab_q.pop(i)
                        i_next = i + n_pre
                        if i_next < IT:
                            slab_q[i_next] = [
                                load_w01_slab("w0", w0t, s, i_next),
                                load_w01_slab("w1", w1t, s, i_next),
                            ]
                        gb = [ps.tile([128, 512], f32, tag="g", bufs=4, name=f"g{jn}_{i}_{ci}") for ci in range(len(chunks))]
                        ub = [ps.tile([128, 512], f32, tag="u", bufs=4, name=f"u{jn}_{i}_{ci}") for ci in range(len(chunks))]
                        for h in range(HT):
                            for ci, (c0, cw) in enumerate(chunks):
                                nc.tensor.matmul(
                                    gb[ci][:, :cw],
                                    w0s[:, ts(h, 128)],
                                    xs[h][:, c0 : c0 + cw],
                                    start=(h == 0),
                                    stop=(h == HT - 1),
                                )
                            for ci, (c0, cw) in enumerate(chunks):
                                nc.tensor.matmul(
                                    ub[ci][:, :cw],
                                    w1s[:, ts(h, 128)],
                                    xs[h][:, c0 : c0 + cw],
                                    start=(h == 0),
                                    stop=(h == HT - 1),
                                )
                        for ci, (c0, cw) in enumerate(chunks):
                            a_sl = acts[i][:, c0 : c0 + cw]
                            nc.scalar.activation(a_sl, gb[ci][:, :cw], act_fn)
                            nc.vector.tensor_mul(a_sl, a_sl, ub[ci][:, :cw])

                    # Phase B: down projection, per h-tile, i-outer with the
                    # same chunk interleaving (PSUM banks shared with the "u"
                    # ring).  y is staged into a full-width SBUF row so the
                    # writeback is one DMA per h with 2.8KB partition lines.
                    for h in range(HT):
                        w2s = wp.tile([128, I], bf16, tag="w2", bufs=3)
                        nc.sync.dma_start(out=w2s, in_=w2t[s, h])
                        yc = yp.tile([128, W], bf16, tag="y", bufs=2)
                        ob = [ps.tile([128, 512], f32, tag="u", bufs=4, name=f"o{jn}_{h}_{ci}") for ci in range(len(chunks))]
                        for i in range(IT):
                            for ci, (c0, cw) in enumerate(chunks):
                                nc.tensor.matmul(
                                    ob[ci][:, :cw],
                                    w2s[:, ts(i, 128)],
                                    acts[i][:, c0 : c0 + cw],
                                    start=(i == 0),
                                    stop=(i == IT - 1),
                                )
                        for ci, (c0, cw) in enumerate(chunks):
                            nc.vector.tensor_copy(yc[:, c0 : c0 + cw], ob[ci][:, :cw])
                        nc.sync.dma_start(
                            out=yt[s, h, :, co : co + W], in_=yc
                        )
    nc.finalize()
    return nc


def _get_built(jobs, CT):
    key = (tuple(jobs), CT)
    if key not in _BUILT:
        _BUILT[key] = _build(tuple(jobs), CT)
    return _BUILT[key]


def _dequant(w, s):
    """w: [E, O, Iin], s: [E, O, Iin//128] -> dequantized [E, O, Iin]."""
    e, o, iin = w.shape
    nb = -(-iin // BS)
    if nb * BS != iin:
        s_full = np.repeat(s, BS, axis=-1)[..., :iin]
        return w * s_full
    return (w.reshape(e, o, nb, BS) * s[..., None]).reshape(e, o, iin)


def _slabify(wd, bf16):
    """wd: [E, O, C] dequantized weights -> [E, O//128, 128, C] bf16 where
    slab o = [128 c-sub partitions, O-tile columns grouped by c-tile]:
    out[e, o, p, ct*128+j] = wd[e, o*128+j, ct*128+p]."""
    e, o, c = wd.shape
    ot, ct = o // 128, c // 128
    v = wd.astype(bf16).reshape(e, ot, 128, ct, 128)
    return v.transpose(0, 1, 4, 3, 2).reshape(e, ot, 128, c)


def kernel(**inputs):
    global LAST_RESULTS
    import ml_dtypes

    bf16 = ml_dtypes.bfloat16

    x = np.ascontiguousarray(np.asarray(inputs["x"], dtype=np.float32))
    sel = np.asarray(inputs["selected_experts"])
    w0 = np.asarray(inputs["w0"], dtype=np.float32)
    s0 = np.asarray(inputs["s0"], dtype=np.float32)
    w1 = np.asarray(inputs["w1"], dtype=np.float32)
    s1 = np.asarray(inputs["s1"], dtype=np.float32)
    w2 = np.asarray(inputs["w2"], dtype=np.float32)
    s2 = np.asarray(inputs["s2"], dtype=np.float32)

    t, k = sel.shape
    assert (t, k) == (T, TOPK) and x.shape == (T, H)

    # ---- host-side dispatch: unique tokens per expert ----
    pos = np.full((E, T), -1, dtype=np.int32)
    cols = []
    for e in range(E):
        toks = np.nonzero((sel == e).any(axis=1))[0]
        cols.append(toks)
        pos[e, toks] = np.arange(len(toks), dtype=np.int32)
    counts = np.array([len(c) for c in cols])

    # Assign experts to (core, slot): slot 0 holds the 8 largest experts,
    # slot 1 the 8 smallest, so each slot's padded width is only the max of
    # its own rank group.  expert_of[s][c] = expert on core c, slot s.
    order = np.argsort(-counts, kind="stable")
    expert_of = [list(order[:NCORES]), list(order[NCORES:])]

    def align8(v):
        return max(256, -(-v // 4) * 4)

    slot_w = [align8(int(counts[expert_of[s]].max())) for s in range(2)]

    if max(slot_w) <= MAX_W:
        jobs = tuple((s, 0, slot_w[s]) for s in range(2))
        CT = max(slot_w)
    else:
        # fallback: uniform width, multiple column windows per slot
        cmax = int(counts.max())
        passes = max(1, math.ceil(cmax / MAX_W))
        W = align8(math.ceil(cmax / passes))
        CT = W * passes
        jobs = tuple((s, cp * W, W) for s in range(2) for cp in range(passes))

    # ---- dequantize + slabify weights (host) ----
    # w0/w1: [E, I, H] -> slabs [E, IT, 128, H]; w2: [E, H, I] -> [E, HT, 128, I]
    w0s_all = _slabify(_dequant(w0, s0), bf16)
    w1s_all = _slabify(_dequant(w1, s1), bf16)
    w2s_all = _slabify(_dequant(w2, s2), bf16)

    x_bf = x.astype(bf16)
    in_maps = []
    for c in range(NCORES):
        pair = [expert_of[0][c], expert_of[1][c]]
        xt_c = np.zeros((2, H, CT), dtype=bf16)
        for s, e in enumerate(pair):
            n = len(cols[e])
            if n:
                xt_c[s, :, :n] = x_bf[cols[e]].T
        in_maps.append(
            {
                "xt": xt_c.reshape(2, HT, 128, CT),
                "w0t": np.ascontiguousarray(w0s_all[pair]),
                "w1t": np.ascontiguousarray(w1s_all[pair]),
                "w2t": np.ascontiguousarray(w2s_all[pair]),
            }
        )

    nc = _get_built(jobs, CT)
    from concourse.bass_utils import run_bass_kernel_spmd

    res = run_bass_kernel_spmd(nc, in_maps, list(range(NCORES)))
    LAST_RESULTS = res

    # Y[e] = [H, CT] for expert e
    Y = np.empty((E, H, CT), dtype=np.float32)
    for c in range(NCORES):
        yt_c = np.asarray(res.results[c]["yt"]).astype(np.float32).reshape(2, H, CT)
        Y[expert_of[0][c]] = yt_c[0]
        Y[expert_of[1][c]] = yt_c[1]

    # ---- scatter back to [T, K, H] ----
    e_flat = sel.reshape(-1).astype(np.int64)
    t_flat = np.repeat(np.arange(T, dtype=np.int64), TOPK)
    p_flat = pos[e_flat, t_flat]
    out = Y[e_flat, :, p_flat]  # [T*K, H]
    return np.ascontiguousarray(out.reshape(T, TOPK, H), dtype=np.float32)


# revision 16
# speedup vs baseline: 1.0013x; 1.0003x over previous
"""DeepseekMoE block-quantized MoE kernel for 8 Trainium2 NeuronCores.

Strategy (expert-parallel with host-side dispatch):
  - The routing table (selected_experts) is known on the host before launch,
    so the all-to-all "dispatch" is done on the host: for each expert e we
    gather the unique tokens routed to it (dedup across the top-k slots),
    transpose to [H, n_e], and pad to a common capacity C.
  - Experts are sharded 2-per-core across the 8 cores.  Each core runs a
    dense 3-matmul MLP (gate/up -> silu*up -> down) for its 2 experts in
    x^T / act^T layout so no on-device transposes are needed.
  - Block-dequantization (w * repeat(s, 128)) is folded into the host-side
    weight preparation, which also emits slab-contiguous weight layouts so
    every weight DMA is a pure linear copy (4KB per partition line).
  - All matmul operands are bf16: same 1 col/cycle PE rate as fp32r, but
    half the HBM traffic (faster pipeline fill, no DMA-induced PE stalls)
    and FWL-eligible LDWEIGHTS (fp32 operands block fast weight load).
  - Loop order is h-outer with chunk-interleaved PSUM accumulation (4-deep
    g/u rings, 8 banks total), so each x h-tile is consumed exactly once at
    DMA delivery pace and consecutive matmuls share the stationary weights.
  - x loads ride the gpsimd queue, weight/output DMAs the sync queue, so
    a job's x prefetch never head-of-line blocks the running job's DMAs.
  - The host scatters the per-expert outputs back to [T, K, H].

  Measured on 8 trn2 cores: 631-633us vs a 594.9us compute-stream floor
  (2704 padded cols x 528 PE cycles/col @ 2.4 GHz); the gap is runtime
  preamble (~7us), bandwidth-structural pipeline fill, and drain tail.
  fp8 was evaluated and rejected: e4m3 activation quantization alone puts
  rel_l2 at ~4.6e-2, over the 2e-2 gate (bf16 lands at 4.7e-3).
"""

import math

import numpy as np

T = 4096
TOPK = 6
E = 16
H = 2048
I = 1408
BS = 128           # quant block size
HT = H // 128      # 16 h-tiles
IT = I // 128      # 11 i-tiles
NCORES = 8
# Single-pass width bound: at most 4 PSUM-bank chunks (the interleaved
# accumulation rings are 4 deep), and (HT + IT) * 2 * W bytes of x+act
# per partition plus ~40KB of staging must fit in ~208KB of SBUF.
MAX_W = 2040

_BUILT = {}
LAST_RESULTS = None  # stashed BassKernelResults for external harnesses


def _chunk_plan(width, small_first=False):
    """Split `width` columns into PSUM-bank-sized chunks (<=512), each >=256
    when width allows (small free dims pay LDWEIGHTS/dispatch overhead).
    With small_first, carve a 256-col leading chunk so the first matmul
    group's input DMA is small (faster pipeline fill at kernel start)."""
    if width <= 512:
        return [(0, width)]
    if small_first and width > 768:
        return [(0, 256)] + [(256 + o, w) for o, w in _chunk_plan(width - 256)]
    n = -(-width // 512)
    # 8-aligned chunk widths
    base = (width // n) // 8 * 8
    rem8 = (width - n * base) // 8
    out, off = [], 0
    for j in range(n):
        w = base + (8 if j < rem8 else 0)
        if j == n - 1:
            w = width - off
        out.append((off, w))
        off += w
    return out


def _solve4(counts):
    """Find 4 job widths [W0..W3] (sum-minimal, 4-aligned) such that for
    every core's expert pair (a, b) some 2+2 partition of the jobs covers
    (a, b).  Each expert's token list is then split across its 2 jobs.
    Returns (widths, cover) where cover[pair_idx] = partition id (0: {0,1}
    vs {2,3}, 1: {0,2} vs {1,3}, 2: {0,3} vs {1,2}), or None."""
    import itertools

    order = np.argsort(-counts, kind="stable")
    pairs = [
        (int(counts[order[i]]), int(counts[order[2 * NCORES - 1 - i]]))
        for i in range(NCORES)
    ]
    best = None
    cands = []
    for assign in itertools.product(range(3), repeat=len(pairs)):
        req = [[0, 0], [0, 0], [0, 0]]
        for (a, b), p in zip(pairs, assign):
            req[p][0] = max(req[p][0], a)
            req[p][1] = max(req[p][1], b)
        lb = max(
            max(ra + rb for ra, rb in req),
            -(-sum(ra + rb for ra, rb in req) // 3),
        )
        cands.append((lb, assign, req))
    cands.sort(key=lambda t: t[0])
    for lb, assign, req in cands[:200]:
        (m1a, m1b), (m2a, m2b), (m3a, m3b) = req
        # A pairs with B / C / D in partitions 1 / 2 / 3 respectively.
        for A in range(256, 1500, 4):
            B = max(m1a - A, 256)
            C = max(m2a - A, m3b - B, 256)
            D = max(m3a - A, m2b - B, 256)
            if C + D < m1b:
                D += m1b - (C + D)
            w = [-(-v // 4) * 4 for v in (A, B, C, D)]
            S = sum(w)
            if best is None or S < best[0]:
                best = (S, w, assign)
        if best is not None and best[0] <= lb:
            break
    if best is None:
        return None
    S, w, assign = best
    parts = {0: ((0, 1), (2, 3)), 1: ((0, 2), (1, 3)), 2: ((0, 3), (1, 2))}
    cover = []
    for (a, b), p in zip(pairs, assign):
        big, small = parts[p]
        if w[big[0]] + w[big[1]] < a or w[small[0]] + w[small[1]] < b:
            return None
        cover.append((big, small))
    experts_of_core = [
        (int(order[i]), int(counts[order[i]]),
         int(order[2 * NCORES - 1 - i]), int(counts[order[2 * NCORES - 1 - i]]))
        for i in range(NCORES)
    ]
    return w, cover, experts_of_core


def _build(jobs, CT):
    """Build the SPMD Bass program.  `jobs` is a tuple of
    (slot, col_offset, width): each job runs one expert slot's MLP over a
    window of `width` token columns; CT is the column capacity of xt/yt."""
    import concourse.bacc as bacc
    import concourse.mybir as mybir
    from concourse.bass import ts
    from concourse.tile import TileContext

    f32 = mybir.dt.float32
    bf16 = mybir.dt.bfloat16
    AF = mybir.ActivationFunctionType
    import os as _os

    act_fn = (
        AF.Sigmoid if _os.environ.get("KERNEL_SIM_SIGMOID") else AF.Silu
    )  # CoreSim lacks Silu; HW path always uses Silu

    NS = max(j[0] for j in jobs) + 1
    nc = bacc.Bacc()
    xt = nc.declare_dram_parameter("xt", [NS, HT, 128, CT], bf16, isOutput=False)
    # slab-contiguous weights: w0t/w1t slab i = [128, H]; w2t slab h = [128, I]
    w0t = nc.declare_dram_parameter("w0t", [NS, IT, 128, H], bf16, isOutput=False)
    w1t = nc.declare_dram_parameter("w1t", [NS, IT, 128, H], bf16, isOutput=False)
    w2t = nc.declare_dram_parameter("w2t", [NS, HT, 128, I], bf16, isOutput=False)
    yt = nc.declare_dram_parameter("yt", [NS, HT, 128, CT], bf16, isOutput=True)

    with TileContext(nc) as tc:
        with (
            tc.tile_pool(name="xp", bufs=1) as xp,
            tc.tile_pool(name="ap", bufs=1) as apool,
            tc.tile_pool(name="wp", bufs=2) as wp,
            tc.tile_pool(name="yp", bufs=4) as yp,
            tc.tile_pool(name="ps", bufs=3, space="PSUM") as ps,
        ):
            def load_w01_slab(which, src, s, i):
                slab = wp.tile([128, H], bf16, tag=which, name=None, bufs=3)
                nc.sync.dma_start(out=slab, in_=src[s, i])
                return slab

            for jn, (s, co, W) in enumerate(jobs):
                    chunks = _chunk_plan(W)
                    xs = [
                        xp.tile([128, CT], bf16, tag=f"x{h}", name=f"x{h}_{jn}")
                        for h in range(HT)
                    ]
                    # Bandwidth-priority emission: weight slabs for i=0..2 on
                    # the sync queue, x h-tiles (full width, 2.8KB DMA lines)
                    # on the gpsimd queue so neither head-of-line blocks the
                    # other.  The PE starts once the i=0 gate slab and the
                    # first x h-tiles land.
                    slab_q = {0: [load_w01_slab("w0", w0t, s, 0)]}
                    for h in range(HT):
                        nc.gpsimd.dma_start(
                            out=xs[h][:, :W], in_=xt[s, h, :, co : co + W]
                        )
                        if h == 0:
                            slab_q[0].append(load_w01_slab("w1", w1t, s, 0))
                        if h <= 1 and h + 1 < IT:
                            slab_q[h + 1] = [
                                load_w01_slab("w0", w0t, s, h + 1),
                                load_w01_slab("w1", w1t, s, h + 1),
                            ]
                    acts = [
                        apool.tile([128, CT], bf16, tag=f"a{i}", name=f"a{i}_{jn}")
                        for i in range(IT)
                    ]

                    # Phase A: gate/up projections + silu*up, per i-tile.
                    # h-outer with chunk-interleaved PSUM accumulation: each
                    # xs[h] is consumed exactly once (streams at DMA delivery
                    # pace during the fill) and consecutive matmuls share the
                    # stationary weight tile.
                    n_pre = max(slab_q) + 1
                    for i in range(IT):
                        w0s, w1s = slab_q.pop(i)
                        i_next = i + n_pre
                        if i_next < IT:
                            slab_q[i_next] = [
                                load_w01_slab("w0", w0t, s, i_next),
                                load_w01_slab("w1", w1t, s, i_next),
                            ]
                        gb = [ps.tile([128, 512], f32, tag="g", bufs=4, name=f"g{jn}_{i}_{ci}") for ci in range(len(chunks))]
                        ub = [ps.tile([128, 512], f32, tag="u", bufs=4, name=f"u{jn}_{i}_{ci}") for ci in range(len(chunks))]
                        for h in range(HT):
                            for ci, (c0, cw) in enumerate(chunks):
                                nc.tensor.matmul(
                                    gb[ci][:, :cw],
                                    w0s[:, ts(h, 128)],
                                    xs[h][:, c0 : c0 + cw],
                                    start=(h == 0),
                                    stop=(h == HT - 1),
                                )
                            for ci, (c0, cw) in enumerate(chunks):
                                nc.tensor.matmul(
                                    ub[ci][:, :cw],
                                    w1s[:, ts(h, 128)],
                                    xs[h][:, c0 : c0 + cw],
                                    start=(h == 0),
                                    stop=(h == HT - 1),
                                )
                        for ci, (c0, cw) in enumerate(chunks):
                            a_sl = acts[i][:, c0 : c0 + cw]
                            nc.scalar.activation(a_sl, gb[ci][:, :cw], act_fn)
                            nc.vector.tensor_mul(a_sl, a_sl, ub[ci][:, :cw])

                    # Phase B: down projection, per h-tile, i-outer with the
                    # same chunk interleaving (PSUM banks shared with the "u"
                    # ring).  y is staged into a full-width SBUF row so the
                    # writeback is one DMA per h with 2.8KB partition lines.
                    for h in range(HT):
                        w2s = wp.tile([128, I], bf16, tag="w2", bufs=3)
                        nc.sync.dma_start(out=w2s, in_=w2t[s, h])
                        yc = yp.tile([128, CT], bf16, tag="y", bufs=2)
                        ob = [ps.tile([128, 512], f32, tag="u", bufs=4, name=f"o{jn}_{h}_{ci}") for ci in range(len(chunks))]
                        for i in range(IT):
                            for ci, (c0, cw) in enumerate(chunks):
                                nc.tensor.matmul(
                                    ob[ci][:, :cw],
                                    w2s[:, ts(i, 128)],
                                    acts[i][:, c0 : c0 + cw],
                                    start=(i == 0),
                                    stop=(i == IT - 1),
                                )
                        for ci, (c0, cw) in enumerate(chunks):
                            nc.vector.tensor_copy(yc[:, c0 : c0 + cw], ob[ci][:, :cw])
                        nc.sync.dma_start(
                            out=yt[s, h, :, co : co + W], in_=yc[:, :W]
                        )
    nc.finalize()
    return nc


def _get_built(jobs, CT):
    key = (tuple(jobs), CT)
    if key not in _BUILT:
        _BUILT[key] = _build(tuple(jobs), CT)
    return _BUILT[key]


def _dequant(w, s):
    """w: [E, O, Iin], s: [E, O, Iin//128] -> dequantized [E, O, Iin]."""
    e, o, iin = w.shape
    nb = -(-iin // BS)
    if nb * BS != iin:
        s_full = np.repeat(s, BS, axis=-1)[..., :iin]
        return w * s_full
    return (w.reshape(e, o, nb, BS) * s[..., None]).reshape(e, o, iin)


def _slabify(wd, bf16):
    """wd: [E, O, C] dequantized weights -> [E, O//128, 128, C] bf16 where
    slab o = [128 c-sub partitions, O-tile columns grouped by c-tile]:
    out[e, o, p, ct*128+j] = wd[e, o*128+j, ct*128+p]."""
    e, o, c = wd.shape
    ot, ct = o // 128, c // 128
    v = wd.astype(bf16).reshape(e, ot, 128, ct, 128)
    return v.transpose(0, 1, 4, 3, 2).reshape(e, ot, 128, c)


def kernel(**inputs):
    global LAST_RESULTS
    import ml_dtypes

    bf16 = ml_dtypes.bfloat16

    x = np.ascontiguousarray(np.asarray(inputs["x"], dtype=np.float32))
    sel = np.asarray(inputs["selected_experts"])
    w0 = np.asarray(inputs["w0"], dtype=np.float32)
    s0 = np.asarray(inputs["s0"], dtype=np.float32)
    w1 = np.asarray(inputs["w1"], dtype=np.float32)
    s1 = np.asarray(inputs["s1"], dtype=np.float32)
    w2 = np.asarray(inputs["w2"], dtype=np.float32)
    s2 = np.asarray(inputs["s2"], dtype=np.float32)

    t, k = sel.shape
    assert (t, k) == (T, TOPK) and x.shape == (T, H)

    # ---- host-side dispatch: unique tokens per expert ----
    pos = np.full((E, T), -1, dtype=np.int32)
    cols = []
    for e in range(E):
        toks = np.nonzero((sel == e).any(axis=1))[0]
        cols.append(toks)
        pos[e, toks] = np.arange(len(toks), dtype=np.int32)
    counts = np.array([len(c) for c in cols])

    def align4(v):
        return max(256, -(-v // 4) * 4)

    order = np.argsort(-counts, kind="stable")
    two_slot_total = align4(int(counts[order[0]])) + align4(int(counts[order[NCORES]]))

    sol4 = _solve4(counts)
    plan = None  # plan[c] = list over slots of (expert, tok_off, ncols)
    if sol4 is not None and sum(sol4[0]) < two_slot_total and max(sol4[0]) <= MAX_W:
        w4, cover, experts_of_core = sol4
        # emit jobs in ascending width order (smallest first: cheaper fill)
        jobs = tuple((int(j), 0, int(w4[j])) for j in np.argsort(w4, kind="stable"))
        CT = max(w4)
        NS = 4
        plan = []
        for c in range(NCORES):
            ea, na, eb, nb = experts_of_core[c]
            big, small = cover[c]
            slots = [None] * NS
            for e, n, jl in ((ea, na, big), (eb, nb, small)):
                n0 = min(n, w4[jl[0]])
                slots[jl[0]] = (e, 0, n0)
                slots[jl[1]] = (e, n0, n - n0)
            plan.append(slots)
    else:
        # 2-slot scheme: slot 0 the 8 largest experts, slot 1 the 8 smallest
        expert_of = [list(order[:NCORES]), list(order[NCORES:])]
        slot_w = [align4(int(counts[expert_of[s]].max())) for s in range(2)]
        NS = 2
        if max(slot_w) <= MAX_W:
            jobs = tuple((s, 0, slot_w[s]) for s in range(2))
            CT = max(slot_w)
        else:
            cmax = int(counts.max())
            passes = max(1, math.ceil(cmax / MAX_W))
            W = align4(math.ceil(cmax / passes))
            CT = W * passes
            jobs = tuple((s, cp * W, W) for s in range(2) for cp in range(passes))
        plan = []
        for c in range(NCORES):
            plan.append(
                [(int(expert_of[s][c]), 0, int(counts[expert_of[s][c]]))
                 for s in range(2)]
            )

    # ---- dequantize + slabify weights (host) ----
    # w0/w1: [E, I, H] -> slabs [E, IT, 128, H]; w2: [E, H, I] -> [E, HT, 128, I]
    w0s_all = _slabify(_dequant(w0, s0), bf16)
    w1s_all = _slabify(_dequant(w1, s1), bf16)
    w2s_all = _slabify(_dequant(w2, s2), bf16)

    x_bf = x.astype(bf16)
    in_maps = []
    for c in range(NCORES):
        xt_c = np.zeros((NS, H, CT), dtype=bf16)
        exps = []
        for s, (e, off, n) in enumerate(plan[c]):
            exps.append(e)
            if n:
                xt_c[s, :, :n] = x_bf[cols[e][off : off + n]].T
        in_maps.append(
            {
                "xt": xt_c.reshape(NS, HT, 128, CT),
                "w0t": np.ascontiguousarray(w0s_all[exps]),
                "w1t": np.ascontiguousarray(w1s_all[exps]),
                "w2t": np.ascontiguousarray(w2s_all[exps]),
            }
        )

    nc = _get_built(jobs, CT)
    from concourse.bass_utils import run_bass_kernel_spmd

    res = run_bass_kernel_spmd(nc, in_maps, list(range(NCORES)))
    LAST_RESULTS = res

    # Y[e] = [H, n_e] for expert e (token order = cols[e])
    Y = np.zeros((E, H, int(counts.max())), dtype=np.float32)
    for c in range(NCORES):
        yt_c = np.asarray(res.results[c]["yt"]).astype(np.float32).reshape(NS, H, CT)
        for s, (e, off, n) in enumerate(plan[c]):
            if n:
                Y[e][:, off : off + n] = yt_c[s][:, :n]

    # ---- scatter back to [T, K, H] ----
    e_flat = sel.reshape(-1).astype(np.int64)
    t_flat = np.repeat(np.arange(T, dtype=np.int64), TOPK)
    p_flat = pos[e_flat, t_flat]
    out = Y[e_flat, :, p_flat]  # [T*K, H]
    return np.ascontiguousarray(out.reshape(T, TOPK, H), dtype=np.float32)


# revision 17
# speedup vs baseline: 1.0082x; 1.0068x over previous
"""DeepseekMoE block-quantized MoE kernel for 8 Trainium2 NeuronCores.

Strategy (expert-parallel with host-side dispatch):
  - The routing table (selected_experts) is known on the host before launch,
    so the all-to-all "dispatch" is done on the host: for each expert e we
    gather the unique tokens routed to it (dedup across the top-k slots),
    transpose to [H, n_e], and pad to a common capacity C.
  - Experts are sharded 2-per-core across the 8 cores.  Each core runs a
    dense 3-matmul MLP (gate/up -> silu*up -> down) for its 2 experts in
    x^T / act^T layout so no on-device transposes are needed.
  - Block-dequantization (w * repeat(s, 128)) is folded into the host-side
    weight preparation, which also emits slab-contiguous weight layouts so
    every weight DMA is a pure linear copy (4KB per partition line).
  - All matmul operands are bf16: same 1 col/cycle PE rate as fp32r, but
    half the HBM traffic (faster pipeline fill, no DMA-induced PE stalls)
    and FWL-eligible LDWEIGHTS (fp32 operands block fast weight load).
  - Loop order is h-outer with chunk-interleaved PSUM accumulation (4-deep
    g/u rings, 8 banks total), so each x h-tile is consumed exactly once at
    DMA delivery pace and consecutive matmuls share the stationary weights.
  - x loads ride the gpsimd queue, weight/output DMAs the sync queue, so
    a job's x prefetch never head-of-line blocks the running job's DMAs.
  - The host scatters the per-expert outputs back to [T, K, H].

  Measured on 8 trn2 cores: 631-633us vs a 594.9us compute-stream floor
  (2704 padded cols x 528 PE cycles/col @ 2.4 GHz); the gap is runtime
  preamble (~7us), bandwidth-structural pipeline fill, and drain tail.
  fp8 was evaluated and rejected: e4m3 activation quantization alone puts
  rel_l2 at ~4.6e-2, over the 2e-2 gate (bf16 lands at 4.7e-3).
"""

import math

import numpy as np

T = 4096
TOPK = 6
E = 16
H = 2048
I = 1408
BS = 128           # quant block size
HT = H // 128      # 16 h-tiles
IT = I // 128      # 11 i-tiles
NCORES = 8
# Single-pass width bound: at most 4 PSUM-bank chunks (the interleaved
# accumulation rings are 4 deep), and (HT + IT) * 2 * W bytes of x+act
# per partition plus ~40KB of staging must fit in ~208KB of SBUF.
MAX_W = 2040

_BUILT = {}
LAST_RESULTS = None  # stashed BassKernelResults for external harnesses


def _chunk_plan(width, small_first=False):
    """Split `width` columns into PSUM-bank-sized chunks (<=512), each >=256
    when width allows (small free dims pay LDWEIGHTS/dispatch overhead).
    With small_first, carve a 256-col leading chunk so the first matmul
    group's input DMA is small (faster pipeline fill at kernel start)."""
    if width <= 512:
        return [(0, width)]
    if small_first and width > 768:
        return [(0, 256)] + [(256 + o, w) for o, w in _chunk_plan(width - 256)]
    n = -(-width // 512)
    # 8-aligned chunk widths
    base = (width // n) // 8 * 8
    rem8 = (width - n * base) // 8
    out, off = [], 0
    for j in range(n):
        w = base + (8 if j < rem8 else 0)
        if j == n - 1:
            w = width - off
        out.append((off, w))
        off += w
    return out


def _solve4(counts):
    """Find 4 job widths [W0..W3] (sum-minimal, 4-aligned) such that for
    every core's expert pair (a, b) some 2+2 partition of the jobs covers
    (a, b).  Each expert's token list is then split across its 2 jobs.
    Returns (widths, cover) where cover[pair_idx] = partition id (0: {0,1}
    vs {2,3}, 1: {0,2} vs {1,3}, 2: {0,3} vs {1,2}), or None."""
    import itertools

    order = np.argsort(-counts, kind="stable")
    pairs = [
        (int(counts[order[i]]), int(counts[order[2 * NCORES - 1 - i]]))
        for i in range(NCORES)
    ]
    best = None
    cands = []
    for assign in itertools.product(range(3), repeat=len(pairs)):
        req = [[0, 0], [0, 0], [0, 0]]
        for (a, b), p in zip(pairs, assign):
            req[p][0] = max(req[p][0], a)
            req[p][1] = max(req[p][1], b)
        lb = max(
            max(ra + rb for ra, rb in req),
            -(-sum(ra + rb for ra, rb in req) // 3),
        )
        cands.append((lb, assign, req))
    cands.sort(key=lambda t: t[0])
    for lb, assign, req in cands[:200]:
        (m1a, m1b), (m2a, m2b), (m3a, m3b) = req
        # A pairs with B / C / D in partitions 1 / 2 / 3 respectively.
        for A in range(256, 1500, 4):
            B = max(m1a - A, 256)
            C = max(m2a - A, m3b - B, 256)
            D = max(m3a - A, m2b - B, 256)
            if C + D < m1b:
                D += m1b - (C + D)
            w = [-(-v // 4) * 4 for v in (A, B, C, D)]
            S = sum(w)
            if best is None or S < best[0]:
                best = (S, w, assign)
        if best is not None and best[0] <= lb:
            break
    if best is None:
        return None
    S, w, assign = best
    parts = {0: ((0, 1), (2, 3)), 1: ((0, 2), (1, 3)), 2: ((0, 3), (1, 2))}
    cover = []
    for (a, b), p in zip(pairs, assign):
        big, small = parts[p]
        if w[big[0]] + w[big[1]] < a or w[small[0]] + w[small[1]] < b:
            return None
        cover.append((big, small))
    experts_of_core = [
        (int(order[i]), int(counts[order[i]]),
         int(order[2 * NCORES - 1 - i]), int(counts[order[2 * NCORES - 1 - i]]))
        for i in range(NCORES)
    ]
    return w, cover, experts_of_core


def _build(jobs, CT):
    """Build the SPMD Bass program.  `jobs` is a tuple of
    (slot, col_offset, width): each job runs one expert slot's MLP over a
    window of `width` token columns; CT is the column capacity of xt/yt."""
    import concourse.bacc as bacc
    import concourse.mybir as mybir
    from concourse.bass import ts
    from concourse.tile import TileContext

    f32 = mybir.dt.float32
    bf16 = mybir.dt.bfloat16
    AF = mybir.ActivationFunctionType
    import os as _os

    act_fn = (
        AF.Sigmoid if _os.environ.get("KERNEL_SIM_SIGMOID") else AF.Silu
    )  # CoreSim lacks Silu; HW path always uses Silu

    NS = max(j[0] for j in jobs) + 1
    nc = bacc.Bacc()
    xt = nc.declare_dram_parameter("xt", [NS, HT, 128, CT], bf16, isOutput=False)
    # slab-contiguous weights: w0t/w1t slab i = [128, H]; w2t slab h = [128, I]
    w0t = nc.declare_dram_parameter("w0t", [NS, IT, 128, H], bf16, isOutput=False)
    w1t = nc.declare_dram_parameter("w1t", [NS, IT, 128, H], bf16, isOutput=False)
    w2t = nc.declare_dram_parameter("w2t", [NS, HT, 128, I], bf16, isOutput=False)
    yt = nc.declare_dram_parameter("yt", [NS, HT, 128, CT], bf16, isOutput=True)

    with TileContext(nc) as tc:
        with (
            tc.tile_pool(name="xp", bufs=1) as xp,
            tc.tile_pool(name="ap", bufs=1) as apool,
            tc.tile_pool(name="wp", bufs=2) as wp,
            tc.tile_pool(name="yp", bufs=4) as yp,
            tc.tile_pool(name="ps", bufs=3, space="PSUM") as ps,
        ):
            def load_w01_slab(which, src, s, i):
                slab = wp.tile([128, H], bf16, tag=which, name=None, bufs=4)
                nc.sync.dma_start(out=slab, in_=src[s, i])
                return slab

            def emit_prefetch(jn):
                # x h-tiles (full width, 2.8KB DMA lines) on the gpsimd
                # queue, weight slabs for i=0..2 on the sync queue, so
                # neither head-of-line blocks the other.  Called one job
                # ahead (before the previous job's DMA-heavy phase B) so
                # the transfers ride the DMA-light phase A window.
                s, co, W = jobs[jn]
                xs = [
                    xp.tile([128, CT], bf16, tag=f"x{h}", name=f"x{h}_{jn}",
                            bufs=2)
                    for h in range(HT)
                ]
                slab_q = {0: [load_w01_slab("w0", w0t, s, 0)]}
                for h in range(HT):
                    nc.gpsimd.dma_start(
                        out=xs[h][:, :W], in_=xt[s, h, :, co : co + W]
                    )
                    if h == 0:
                        slab_q[0].append(load_w01_slab("w1", w1t, s, 0))
                    if h <= 1 and h + 1 < IT:
                        slab_q[h + 1] = [
                            load_w01_slab("w0", w0t, s, h + 1),
                            load_w01_slab("w1", w1t, s, h + 1),
                        ]
                return xs, slab_q

            pre = emit_prefetch(0)
            for jn, (s, co, W) in enumerate(jobs):
                    chunks = _chunk_plan(W)
                    xs, slab_q = pre
                    acts = [
                        apool.tile([128, CT], bf16, tag=f"a{i}", name=f"a{i}_{jn}")
                        for i in range(IT)
                    ]

                    # Phase A: gate/up projections + silu*up, per i-tile.
                    # h-outer with chunk-interleaved PSUM accumulation: each
                    # xs[h] is consumed exactly once (streams at DMA delivery
                    # pace during the fill) and consecutive matmuls share the
                    # stationary weight tile.
                    n_pre = max(slab_q) + 1
                    for i in range(IT):
                        w0s, w1s = slab_q.pop(i)
                        i_next = i + n_pre
                        if i_next < IT:
                            slab_q[i_next] = [
                                load_w01_slab("w0", w0t, s, i_next),
                                load_w01_slab("w1", w1t, s, i_next),
                            ]
                        gb = [ps.tile([128, 512], f32, tag="g", bufs=4, name=f"g{jn}_{i}_{ci}") for ci in range(len(chunks))]
                        ub = [ps.tile([128, 512], f32, tag="u", bufs=4, name=f"u{jn}_{i}_{ci}") for ci in range(len(chunks))]
                        for h in range(HT):
                            for ci, (c0, cw) in enumerate(chunks):
                                nc.tensor.matmul(
                                    gb[ci][:, :cw],
                                    w0s[:, ts(h, 128)],
                                    xs[h][:, c0 : c0 + cw],
                                    start=(h == 0),
                                    stop=(h == HT - 1),
                                )
                            for ci, (c0, cw) in enumerate(chunks):
                                nc.tensor.matmul(
                                    ub[ci][:, :cw],
                                    w1s[:, ts(h, 128)],
                                    xs[h][:, c0 : c0 + cw],
                                    start=(h == 0),
                                    stop=(h == HT - 1),
                                )
                        for ci, (c0, cw) in enumerate(chunks):
                            a_sl = acts[i][:, c0 : c0 + cw]
                            nc.scalar.activation(a_sl, gb[ci][:, :cw], act_fn)
                            nc.vector.tensor_mul(a_sl, a_sl, ub[ci][:, :cw])

                    if jn + 1 < len(jobs):
                        pre = emit_prefetch(jn + 1)

                    # Phase B: down projection, per h-tile, i-outer with the
                    # same chunk interleaving (PSUM banks shared with the "u"
                    # ring).  y is staged into a full-width SBUF row so the
                    # writeback is one DMA per h with 2.8KB partition lines.
                    for h in range(HT):
                        w2s = wp.tile([128, I], bf16, tag="w2", bufs=3)
                        nc.sync.dma_start(out=w2s, in_=w2t[s, h])
                        yc = yp.tile([128, CT], bf16, tag="y", bufs=2)
                        ob = [ps.tile([128, 512], f32, tag="u", bufs=4, name=f"o{jn}_{h}_{ci}") for ci in range(len(chunks))]
                        for i in range(IT):
                            for ci, (c0, cw) in enumerate(chunks):
                                nc.tensor.matmul(
                                    ob[ci][:, :cw],
                                    w2s[:, ts(i, 128)],
                                    acts[i][:, c0 : c0 + cw],
                                    start=(i == 0),
                                    stop=(i == IT - 1),
                                )
                        for ci, (c0, cw) in enumerate(chunks):
                            nc.vector.tensor_copy(yc[:, c0 : c0 + cw], ob[ci][:, :cw])
                        nc.sync.dma_start(
                            out=yt[s, h, :, co : co + W], in_=yc[:, :W]
                        )
    nc.finalize()
    return nc


def _get_built(jobs, CT):
    key = (tuple(jobs), CT)
    if key not in _BUILT:
        _BUILT[key] = _build(tuple(jobs), CT)
    return _BUILT[key]


def _dequant(w, s):
    """w: [E, O, Iin], s: [E, O, Iin//128] -> dequantized [E, O, Iin]."""
    e, o, iin = w.shape
    nb = -(-iin // BS)
    if nb * BS != iin:
        s_full = np.repeat(s, BS, axis=-1)[..., :iin]
        return w * s_full
    return (w.reshape(e, o, nb, BS) * s[..., None]).reshape(e, o, iin)


def _slabify(wd, bf16):
    """wd: [E, O, C] dequantized weights -> [E, O//128, 128, C] bf16 where
    slab o = [128 c-sub partitions, O-tile columns grouped by c-tile]:
    out[e, o, p, ct*128+j] = wd[e, o*128+j, ct*128+p]."""
    e, o, c = wd.shape
    ot, ct = o // 128, c // 128
    v = wd.astype(bf16).reshape(e, ot, 128, ct, 128)
    return v.transpose(0, 1, 4, 3, 2).reshape(e, ot, 128, c)


def kernel(**inputs):
    global LAST_RESULTS
    import ml_dtypes

    bf16 = ml_dtypes.bfloat16

    x = np.ascontiguousarray(np.asarray(inputs["x"], dtype=np.float32))
    sel = np.asarray(inputs["selected_experts"])
    w0 = np.asarray(inputs["w0"], dtype=np.float32)
    s0 = np.asarray(inputs["s0"], dtype=np.float32)
    w1 = np.asarray(inputs["w1"], dtype=np.float32)
    s1 = np.asarray(inputs["s1"], dtype=np.float32)
    w2 = np.asarray(inputs["w2"], dtype=np.float32)
    s2 = np.asarray(inputs["s2"], dtype=np.float32)

    t, k = sel.shape
    assert (t, k) == (T, TOPK) and x.shape == (T, H)

    # ---- host-side dispatch: unique tokens per expert ----
    pos = np.full((E, T), -1, dtype=np.int32)
    cols = []
    for e in range(E):
        toks = np.nonzero((sel == e).any(axis=1))[0]
        cols.append(toks)
        pos[e, toks] = np.arange(len(toks), dtype=np.int32)
    counts = np.array([len(c) for c in cols])

    def align4(v):
        return max(256, -(-v // 4) * 4)

    order = np.argsort(-counts, kind="stable")
    two_slot_total = align4(int(counts[order[0]])) + align4(int(counts[order[NCORES]]))

    sol4 = _solve4(counts)
    plan = None  # plan[c] = list over slots of (expert, tok_off, ncols)
    if sol4 is not None and sum(sol4[0]) < two_slot_total and max(sol4[0]) <= MAX_W:
        w4, cover, experts_of_core = sol4
        # emit jobs in ascending width order (smallest first: cheaper fill)
        jobs = tuple((int(j), 0, int(w4[j])) for j in np.argsort(w4, kind="stable"))
        CT = max(w4)
        NS = 4
        plan = []
        for c in range(NCORES):
            ea, na, eb, nb = experts_of_core[c]
            big, small = cover[c]
            slots = [None] * NS
            for e, n, jl in ((ea, na, big), (eb, nb, small)):
                n0 = min(n, w4[jl[0]])
                slots[jl[0]] = (e, 0, n0)
                slots[jl[1]] = (e, n0, n - n0)
            plan.append(slots)
    else:
        # 2-slot scheme: slot 0 the 8 largest experts, slot 1 the 8 smallest
        expert_of = [list(order[:NCORES]), list(order[NCORES:])]
        slot_w = [align4(int(counts[expert_of[s]].max())) for s in range(2)]
        NS = 2
        if max(slot_w) <= MAX_W:
            jobs = tuple((s, 0, slot_w[s]) for s in range(2))
            CT = max(slot_w)
        else:
            cmax = int(counts.max())
            passes = max(1, math.ceil(cmax / MAX_W))
            W = align4(math.ceil(cmax / passes))
            CT = W * passes
            jobs = tuple((s, cp * W, W) for s in range(2) for cp in range(passes))
        plan = []
        for c in range(NCORES):
            plan.append(
                [(int(expert_of[s][c]), 0, int(counts[expert_of[s][c]]))
                 for s in range(2)]
            )

    # ---- dequantize + slabify weights (host) ----
    # w0/w1: [E, I, H] -> slabs [E, IT, 128, H]; w2: [E, H, I] -> [E, HT, 128, I]
    w0s_all = _slabify(_dequant(w0, s0), bf16)
    w1s_all = _slabify(_dequant(w1, s1), bf16)
    w2s_all = _slabify(_dequant(w2, s2), bf16)

    x_bf = x.astype(bf16)
    in_maps = []
    for c in range(NCORES):
        xt_c = np.zeros((NS, H, CT), dtype=bf16)
        exps = []
        for s, (e, off, n) in enumerate(plan[c]):
            exps.append(e)
            if n:
                xt_c[s, :, :n] = x_bf[cols[e][off : off + n]].T
        in_maps.append(
            {
                "xt": xt_c.reshape(NS, HT, 128, CT),
                "w0t": np.ascontiguousarray(w0s_all[exps]),
                "w1t": np.ascontiguousarray(w1s_all[exps]),
                "w2t": np.ascontiguousarray(w2s_all[exps]),
            }
        )

    nc = _get_built(jobs, CT)
    from concourse.bass_utils import run_bass_kernel_spmd

    res = run_bass_kernel_spmd(nc, in_maps, list(range(NCORES)))
    LAST_RESULTS = res

    # Y[e] = [H, n_e] for expert e (token order = cols[e])
    Y = np.zeros((E, H, int(counts.max())), dtype=np.float32)
    for c in range(NCORES):
        yt_c = np.asarray(res.results[c]["yt"]).astype(np.float32).reshape(NS, H, CT)
        for s, (e, off, n) in enumerate(plan[c]):
            if n:
                Y[e][:, off : off + n] = yt_c[s][:, :n]

    # ---- scatter back to [T, K, H] ----
    e_flat = sel.reshape(-1).astype(np.int64)
    t_flat = np.repeat(np.arange(T, dtype=np.int64), TOPK)
    p_flat = pos[e_flat, t_flat]
    out = Y[e_flat, :, p_flat]  # [T*K, H]
    return np.ascontiguousarray(out.reshape(T, TOPK, H), dtype=np.float32)
